# revision 20
# baseline (speedup 1.0000x reference)
"""Distributed GQA attention kernel for 8 TRN2 NeuronCores.

Strategy (tensor-parallel over heads, A2A re-shard before o_proj):
  - Core i owns q heads 4i..4i+3 and kv head i (GQA group) -> attention is
    fully local per core.
  - QKV projection computed TRANSPOSED (features on partitions):
      qkvT[f, s] = w_qkv_shard @ hidden.T   (lhsT = w_qkv_shard.T, rhs = hidden.T)
    Emitted in two batches: batch A (kv block + first half of q pair 0) runs
    kt-major across 6 PSUM banks so the matmul stream paces with the hT DMA
    stream and the PE stays HAM-warm; batch B (rest) is interleaved into the
    attention emission so the PE fills ACT-bound slack.
  - RoPE (neox) applied in [d, s] layout via bf16 DVE ops (2x mode).
  - Attention in "scoresT" layout: scoresT[k, q] = kT.T @ qT, softmax without
    max-subtraction; exp on ACT; causal masking via a single [128,128]
    triangular mask multiply per diagonal 128-block after exp; PV matmul with
    ones-augmented v gives ctxT plus softmax denominators in one accumulation.
    The kb loop is software-pipelined (scores(kb+1) issues before PV(kb)) so
    the PE FIFO never stalls waiting for exp; diagonal blocks are trimmed to
    their live query columns, which cuts exp/scores/PV work ~15%.
  - Two AllToAlls (one per head pair) re-shard ctxT from head-sharded to
    sequence-sharded; the first overlaps the second pair's attention; a tiny
    warm-up collective at kernel start absorbs the ncfw cold-start delay.
    w_o streams in halves (even kt at kernel start, odd kt mid-attention) so
    its HBM traffic never competes with the collectives.
  - o_proj locally on own 256 seq rows with full w_o.T, split into an even-kt
    pass (only needs the first A2A) that hides under the second A2A, then an
    odd-kt pass. Output row-sharded, host concatenates.
"""

import os
import numpy as np
import ml_dtypes

import concourse.bass as bass
import concourse.mybir as mybir
from concourse import bacc, tile
import bass_rust as _br

F32 = mybir.dt.float32
BF16 = mybir.dt.bfloat16
BF16_NP = ml_dtypes.bfloat16

# Problem constants (hardcoded per harness contract)
H = 2048
S = 2048
NH = 32
NKV = 8
HD = 64
Q_SIZE = NH * HD      # 2048
KV_SIZE = NKV * HD    # 512
NCORES = 8
QH = NH // NCORES     # 4 q heads per core
ROPE_THETA = 10000.0

P = 128
KT = H // P           # 16 contraction tiles over H
NQ = 512              # query chunk (matmul moving free dim)
NJC = S // NQ         # 4 query chunks
NKB = S // P          # 16 key tiles
SROWS = S // NCORES   # 256 seq rows per core after A2A

_NC_CACHE = None
LAST_RESULTS = None


def _build_nc():
    nc = bacc.Bacc(
        "TRN2",
        target_bir_lowering=False,
        debug=False,
        num_devices=NCORES,
    )

    # ---- I/O ----
    hT_d = nc.dram_tensor("hT", [P, KT * S], BF16, kind="ExternalInput")
    wq_d = nc.dram_tensor("wqkvT", [P, KT * 384], BF16, kind="ExternalInput")
    wo_d = nc.dram_tensor("woT", [P, KT * H], BF16, kind="ExternalInput")
    ropec_d = nc.dram_tensor("ropec", [P, S], BF16, kind="ExternalInput")
    ropes_d = nc.dram_tensor("ropes", [P, S], BF16, kind="ExternalInput")
    masks_d = nc.dram_tensor("masks", [P, P], BF16, kind="ExternalInput")
    ident_d = nc.dram_tensor("ident", [P, P], BF16, kind="ExternalInput")
    out_d = nc.dram_tensor("out", [SROWS, H], BF16, kind="ExternalOutput")

    rg = [list(range(NCORES))]

    with tile.TileContext(nc) as tc:
        with (
            tc.tile_pool(name="dram", bufs=1, space="DRAM") as dram,
            tc.tile_pool(name="const", bufs=1) as const,
            tc.tile_pool(name="qk", bufs=1) as qkpool,
            tc.tile_pool(name="esb", bufs=3) as esb,
            tc.tile_pool(name="small", bufs=2) as small,
            tc.tile_pool(name="outp", bufs=2) as outp,
            tc.tile_pool(name="ropetmp", bufs=1) as ropetmp,
            tc.tile_pool(name="woev", bufs=1) as woev,
        ):
            # A2A buffers, one per head pair (shard j rows = my pair ctxT for
            # seq cols of core j)
            cc_in = [
                dram.tile([NCORES * P, SROWS], BF16, tag=f"cc_in{p}", name=f"cc_in{p}")
                for p in range(2)
            ]
            cc_out = [
                dram.tile([NCORES * P, SROWS], BF16, tag=f"cc_out{p}", name=f"cc_out{p}")
                for p in range(2)
            ]

            # tiny warm-up collective absorbs the ~11.5us ncfw first-collective
            # start delay so the real A2As get the warm path
            ccw_in = dram.tile([NCORES, 16], BF16, tag="ccw_in")
            ccw_out = dram.tile([NCORES, 16], BF16, tag="ccw_out")
            warm_sb = const.tile([NCORES, 16], BF16, tag="warm_sb")
            nc.vector.memset(warm_sb[:], 0.0)
            nc.sync.dma_start(ccw_in[:], warm_sb[:])
            nc.gpsimd.collective_compute(
                "AllToAll",
                mybir.AluOpType.bypass,
                replica_groups=rg,
                ins=[ccw_in.opt()],
                outs=[ccw_out.opt()],
            )

            qpair = [
                qkpool.tile([P, S], BF16, tag=f"qpair{m}", name=f"qpair{m}")
                for m in range(2)
            ]
            kdup = qkpool.tile([P, S], BF16, tag="kdup")
            v_aug = qkpool.tile([P, NKB * 65], BF16, tag="v_aug")
            cc_sb = qkpool.tile([P, KT * SROWS], BF16, tag="cc_sb")
            qkv_sb = qkpool.tile([P, 3 * S], BF16, tag="qkv_sb")
            wo_even = woev.tile([P, 8 * H], BF16, tag="wo_even")

            # ========== Phase 1: DMAs + QKV batch A ==========
            ph1_ctx = tc.tile_pool(name="ph1", bufs=1)
            ph1 = ph1_ctx.__enter__()
            # DMA priority: weights + hidden k-tiles first (feed PE),
            # rope/mask constants after, then the even-kt half of w_o.
            wq_sb = ph1.tile([P, KT * 384], BF16, tag="wq_sb")
            h_tiles = []
            ropec = ropes = ident = masks = None
            for kt in range(KT):
                nc.sync.dma_start(
                    wq_sb[:, kt * 384 : (kt + 1) * 384],
                    wq_d[:, kt * 384 : (kt + 1) * 384],
                )
                ht = ph1.tile([P, S], BF16, tag=f"h{kt}", name=f"h{kt}")
                nc.sync.dma_start(ht[:], hT_d[:, kt * S : (kt + 1) * S])
                h_tiles.append(ht)
                if kt == 7:
                    # rope/mask constants mid-stream: early enough for the
                    # batch-A rope chunks, without delaying the first h tiles
                    ropec = const.tile([P, S], BF16, tag="ropec")
                    nc.sync.dma_start(ropec[:], ropec_d[:])
                    ropes = const.tile([P, S], BF16, tag="ropes")
                    nc.sync.dma_start(ropes[:], ropes_d[:])
                    ident = const.tile([P, P], BF16, tag="ident")
                    nc.sync.dma_start(ident[:], ident_d[:])
                    masks = const.tile([P, P], BF16, tag="masks")
                    nc.sync.dma_start(masks[:], masks_d[:])

            def emit_rope_q(m, n):
                # roped q for pair m, one NQ chunk; bf16 DVE ops (2x mode)
                c0, c1 = m * S + NQ * n, m * S + NQ * (n + 1)
                swp = ropetmp.tile([P, NQ], BF16, tag="swp", name="swp", bufs=2)
                for b in range(2):
                    o = 64 * b
                    nc.vector.tensor_copy(
                        swp[o : o + 32, :], qkv_sb[o + 32 : o + 64, c0:c1]
                    )
                    nc.vector.tensor_copy(
                        swp[o + 32 : o + 64, :], qkv_sb[o : o + 32, c0:c1]
                    )
                prod = ropetmp.tile([P, NQ], BF16, tag="prod", name="prod", bufs=2)
                nc.vector.tensor_mul(
                    prod[:], qkv_sb[:, c0:c1], ropec[:, NQ * n : NQ * (n + 1)]
                )
                prod2 = ropetmp.tile([P, NQ], BF16, tag="prod2", name="prod2", bufs=2)
                nc.vector.tensor_mul(
                    prod2[:], swp[:], ropes[:, NQ * n : NQ * (n + 1)]
                )
                nc.vector.tensor_add(
                    qpair[m][:, NQ * n : NQ * (n + 1)], prod[:], prod2[:]
                )

            def emit_rope_k(n):
                # rope k (partitions 0..63 of kv block) + duplicate to 64..127
                c0 = 2 * S + NQ * n
                c1 = 2 * S + NQ * (n + 1)
                swpk = ropetmp.tile([P, NQ], BF16, tag="swp", name="swp", bufs=2)
                nc.vector.tensor_copy(swpk[0:32, :], qkv_sb[32:64, c0:c1])
                nc.vector.tensor_copy(swpk[32:64, :], qkv_sb[0:32, c0:c1])
                prodk = ropetmp.tile([P, NQ], BF16, tag="prod", name="prod", bufs=2)
                nc.vector.tensor_mul(
                    prodk[0:64, :], qkv_sb[0:64, c0:c1],
                    ropec[0:64, NQ * n : NQ * (n + 1)],
                )
                prodk2 = ropetmp.tile([P, NQ], BF16, tag="prod2", name="prod2", bufs=2)
                nc.vector.tensor_mul(
                    prodk2[0:64, :], swpk[0:64, :],
                    ropes[0:64, NQ * n : NQ * (n + 1)],
                )
                nc.vector.tensor_add(
                    kdup[0:64, NQ * n : NQ * (n + 1)],
                    prodk[0:64, :], prodk2[0:64, :],
                )
                nc.vector.tensor_add(
                    kdup[64:128, NQ * n : NQ * (n + 1)],
                    prodk[0:64, :], prodk2[0:64, :],
                )

            # Batch A: kv block (all n) + q pair 0 (n=0,1), kt-major over 6
            # PSUM banks so the matmul stream paces with the hT DMA arrival
            # and nothing serializes behind a single accumulation group.
            A_GROUPS = [(2, 0), (2, 1), (2, 2), (2, 3), (0, 0), (0, 1)]
            psa_ctx = tc.tile_pool(name="ps_a", bufs=1, space="PSUM")
            ps_a = psa_ctx.__enter__()
            a_ps = {
                (m, n): ps_a.tile([P, NQ], F32, tag=f"a{m}{n}", name=f"a{m}{n}")
                for (m, n) in A_GROUPS
            }
            for kt in range(KT):
                for (m, n) in A_GROUPS:
                    nc.tensor.matmul(
                        a_ps[(m, n)][:],
                        wq_sb[:, kt * 384 + 128 * m : kt * 384 + 128 * (m + 1)],
                        h_tiles[kt][:, NQ * n : NQ * (n + 1)],
                        start=(kt == 0),
                        stop=(kt == KT - 1),
                    )
            # PSUM -> SBUF bf16 on ACT (idle until attention starts); the
            # attention-critical chunks (n=0 k/q) lead so the first scores
            # matmul isn't gated behind the whole copy burst
            for (m, n) in [(2, 0), (0, 0), (2, 1), (0, 1), (2, 2), (2, 3)]:
                nc.scalar.copy(
                    qkv_sb[:, m * S + NQ * n : m * S + NQ * (n + 1)],
                    a_ps[(m, n)][:],
                )
            psa_ctx.__exit__(None, None, None)
            # rope for the first k/q chunks immediately (gates exp start);
            # the rest trail in the attention filler
            emit_rope_k(0)
            emit_rope_q(0, 0)

            # ========== Phase 2: attention + QKV batch B interleaved ==========
            cc_insts = []
            last_ccin = [None, None]
            ps_b_ctx = tc.tile_pool(name="ps_b", bufs=1, space="PSUM")
            ps_b = ps_b_ctx.__enter__()
            ps_s_ctx = tc.tile_pool(name="ps_s", bufs=2, space="PSUM")
            ps_s = ps_s_ctx.__enter__()
            ps_ctx_ctx = tc.tile_pool(name="ps_ctx", bufs=1, space="PSUM")
            ps_ctx = ps_ctx_ctx.__enter__()

            def emit_vtrans(kb):
                # vT[dv, keys] -> v_aug[keys, dv | 1]; bf16 transpose output
                # must match the bf16 input dtype
                vps = ps_b.tile([P, 64], BF16, tag="vt", name="vt", bufs=1)
                nc.tensor.transpose(
                    vps[:, 0:64],
                    qkv_sb[64:128, 2 * S + P * kb : 2 * S + P * (kb + 1)],
                    ident[64:128, 64:128],
                )
                nc.vector.tensor_copy(
                    v_aug[:, kb * 65 : kb * 65 + 64], vps[:, 0:64]
                )
                nc.vector.memset(v_aug[:, kb * 65 + 64 : kb * 65 + 65], 1.0)

            def b_group_thunks(m, n):
                # batch-B QKV group as 17 filler thunks (16 accumulating
                # matmuls + copy/rope), shared PSUM tile via closure state
                state = {}

                def mk_mm(kt):
                    def f():
                        if kt == 0:
                            state["ps"] = ps_b.tile([P, NQ], F32, tag="b", name="b")
                        nc.tensor.matmul(
                            state["ps"][:],
                            wq_sb[:, kt * 384 + 128 * m : kt * 384 + 128 * (m + 1)],
                            h_tiles[kt][:, NQ * n : NQ * (n + 1)],
                            start=(kt == 0),
                            stop=(kt == KT - 1),
                        )
                    return f

                def fin():
                    nc.vector.tensor_copy(
                        qkv_sb[:, m * S + NQ * n : m * S + NQ * (n + 1)],
                        state["ps"][:],
                    )
                    emit_rope_q(m, n)

                return [mk_mm(kt) for kt in range(KT)] + [fin]

            def emit_attn(p, jc, filler=None, per_kb=3):
                # software-pipelined kb loop: scores(kb+1) issues before
                # PV(kb) so the PE FIFO never stalls on exp; diagonal blocks
                # trimmed to live query columns. `filler` is a list of
                # emission thunks (batch-B QKV matmuls, v transposes) popped
                # a few per kb into the slot where the PE would otherwise
                # wait on exp -- keeps the PE dense and HAM-warm without
                # starving the ACT exp stream.
                nkb = 4 * (jc + 1)
                ctxs = [
                    ps_ctx.tile([P, NQ], F32, tag=f"ctx{hh}", name=f"ctx{hh}")
                    for hh in range(2)
                ]
                pending = None  # (kb, e_tile, lo)

                def emit_pv(kb, e, lo):
                    for hh in range(2):
                        nc.tensor.matmul(
                            ctxs[hh][0:65, lo:NQ],
                            v_aug[:, kb * 65 : kb * 65 + 65],
                            e[:, NQ * hh + lo : NQ * (hh + 1)],
                            start=(kb == 0),
                            stop=(kb == nkb - 1),
                        )

                for kb in range(nkb):
                    d = kb - 4 * jc
                    lo = 128 * d if d > 0 else 0
                    sp = ps_s.tile([P, 2 * NQ], F32, tag="sp", name="sp")
                    for hh in range(2):
                        base = 64 * hh
                        nc.tensor.matmul(
                            sp[:, NQ * hh + lo : NQ * (hh + 1)],
                            kdup[base : base + 64, P * kb : P * (kb + 1)],
                            qpair[p][base : base + 64, NQ * jc + lo : NQ * (jc + 1)],
                            start=True,
                            stop=True,
                        )
                    e = esb.tile([P, 2 * NQ], BF16, tag="e", name="e")
                    if lo == 0:
                        nc.scalar.activation(
                            e[:], sp[:], mybir.ActivationFunctionType.Exp,
                            scale=0.125,
                        )
                    else:
                        nc.scalar.activation(
                            e[:].rearrange("p (h w) -> p h w", h=2)[:, :, lo:NQ],
                            sp[:].rearrange("p (h w) -> p h w", h=2)[:, :, lo:NQ],
                            mybir.ActivationFunctionType.Exp,
                            scale=0.125,
                        )
                    if d >= 0:
                        for hh in range(2):
                            c0 = NQ * hh + lo
                            nc.vector.tensor_mul(
                                e[:, c0 : c0 + P], e[:, c0 : c0 + P], masks[:]
                            )
                    if filler:
                        for _ in range(min(per_kb, len(filler))):
                            filler.pop(0)()
                    if pending is not None:
                        emit_pv(*pending)
                    pending = (kb, e, lo)
                emit_pv(*pending)

                # normalize by the softmax denominators (row 64 of ctx PSUM)
                # and scatter to the A2A input buffer
                for hh in range(2):
                    # reciprocal_approx_fast misreads PSUM at nonzero base
                    # partition -- stage the sum row through SBUF
                    rin = small.tile([1, NQ], F32, tag="rin")
                    nc.vector.tensor_copy(rin[:], ctxs[hh][64:65, :])
                    rec = small.tile([1, NQ], F32, tag="rec")
                    nc.vector.reciprocal_approx_fast(rec[:], rin[:])
                    bcs = small.tile([64, NQ], F32, tag="bcs")
                    nc.gpsimd.partition_broadcast(bcs[:], rec[:], channels=64)
                    ctxn = small.tile([64, NQ], BF16, tag="ctxn")
                    nc.vector.tensor_mul(ctxn[:], ctxs[hh][0:64, :], bcs[:])
                    # scatter: shard j (rows 128j..) holds my pair-p ctxT
                    # rows [64*hh ..] for core j's seq cols
                    for half in range(2):
                        j = 2 * jc + half
                        last_ccin[p] = nc.sync.dma_start(
                            cc_in[p][P * j + 64 * hh : P * j + 64 * (hh + 1), :],
                            ctxn[:, SROWS * half : SROWS * (half + 1)],
                        )

            # interleave: attention(pair 0) with the trailing rope chunks,
            # vtrans, and batch B groups, popped a few per kb so the PE
            # fills ACT-bound slack without ever starving the exp stream
            filler = [
                lambda: emit_rope_k(1),
                lambda: emit_rope_q(0, 1),
            ]
            filler += [(lambda kb=kb: emit_vtrans(kb)) for kb in range(4, 8)]
            filler += [lambda: emit_rope_k(2), lambda: emit_rope_k(3)]
            filler += [(lambda kb=kb: emit_vtrans(kb)) for kb in range(8, 16)]
            for (m, n) in [(0, 2), (0, 3), (1, 0), (1, 1), (1, 2), (1, 3)]:
                filler += b_group_thunks(m, n)
            for kb in range(4):
                emit_vtrans(kb)
            emit_attn(0, 0, filler, per_kb=2)
            emit_attn(0, 1, filler, per_kb=4)
            # the even-kt half of w_o streams during mid pair-0 attention
            # (DMA slack; clear of both the hT stream and the A2As)
            nc.sync.dma_start(
                wo_even[:].rearrange("p (kt c) -> p kt c", kt=8),
                wo_d[:].rearrange("p (kt c) -> p kt c", kt=KT)[:, 0:KT:2, :],
            )
            emit_attn(0, 2, filler, per_kb=5)
            while filler:
                filler.pop(0)()
            # batch B done: release h/wq SBUF, stream the odd-kt half of w_o
            # during the last pair-0 chunk (finishes before the first A2A)
            ph1_ctx.__exit__(None, None, None)
            wood_ctx = tc.tile_pool(name="wood", bufs=1)
            wood = wood_ctx.__enter__()
            wo_odd = wood.tile([P, 8 * H], BF16, tag="wo_odd")
            nc.sync.dma_start(
                wo_odd[:].rearrange("p (kt c) -> p kt c", kt=8),
                wo_d[:].rearrange("p (kt c) -> p kt c", kt=KT)[:, 1:KT:2, :],
            )
            emit_attn(0, 3)
            cc_insts.append(
                nc.gpsimd.collective_compute(
                    "AllToAll",
                    mybir.AluOpType.bypass,
                    replica_groups=rg,
                    ins=[cc_in[0].opt()],
                    outs=[cc_out[0].opt()],
                )
            )
            for jc in range(NJC):
                emit_attn(1, jc)
            cc_insts.append(
                nc.gpsimd.collective_compute(
                    "AllToAll",
                    mybir.AluOpType.bypass,
                    replica_groups=rg,
                    ins=[cc_in[1].opt()],
                    outs=[cc_out[1].opt()],
                )
            )
            ps_ctx_ctx.__exit__(None, None, None)
            ps_s_ctx.__exit__(None, None, None)
            ps_b_ctx.__exit__(None, None, None)

            # ========== Phase 3: o_proj on own seq rows ==========
            # qd chunk (2j + p) <- cc_out[p] rows [128j .. 128j+128)
            for p in range(2):
                for j in range(NCORES):
                    kt = 2 * j + p
                    dma = nc.sync.dma_start(
                        cc_sb[:, kt * SROWS : (kt + 1) * SROWS],
                        cc_out[p][j * P : (j + 1) * P, :],
                    )
                    # prevent Sync-queue head-of-line blocking: these DMAs
                    # wait on collective completion, so don't let the
                    # scheduler hoist them ahead of that pair's scatter
                    # traffic. Pair-0 gathers only pin behind pair-0
                    # scatters so the even o_proj pass starts as soon as
                    # the first AllToAll lands.
                    _br.add_dep_helper(
                        dma.ins, last_ccin[p].ins, sync=True,
                        reason="cc_sb read after scatter traffic",
                    )
            with (
                tc.tile_pool(name="ps_o", bufs=1, space="PSUM") as ps_o,
            ):
                # 8 PSUM banks, one per (n, m); even-kt chunks only need
                # cc_out[0] so this pass hides under the second AllToAll.
                wo_half = [wo_even, wo_odd]
                o_ps = {}
                for n in range(NJC):
                    for m in range(2):
                        o_ps[(n, m)] = ps_o.tile(
                            [P, NQ], F32, tag=f"o{n}{m}", name=f"o{n}{m}"
                        )
                for parity in range(2):
                    for n in range(NJC):
                        for m in range(2):
                            for kk in range(KT // 2):
                                kt = 2 * kk + parity
                                nc.tensor.matmul(
                                    o_ps[(n, m)][:],
                                    cc_sb[
                                        :, kt * SROWS + P * m : kt * SROWS + P * (m + 1)
                                    ],
                                    wo_half[parity][
                                        :, kk * H + NQ * n : kk * H + NQ * (n + 1)
                                    ],
                                    start=(parity == 0 and kk == 0),
                                    stop=(parity == 1 and kk == KT // 2 - 1),
                                )
                            if parity == 1:
                                ot = outp.tile([P, NQ], BF16, tag="ot")
                                nc.vector.tensor_copy(ot[:], o_ps[(n, m)][:])
                                nc.sync.dma_start(
                                    out_d[P * m : P * (m + 1), NQ * n : NQ * (n + 1)],
                                    ot[:],
                                )
            wood_ctx.__exit__(None, None, None)

    nc.compile()
    return nc


def _get_nc():
    global _NC_CACHE
    if _NC_CACHE is None:
        _NC_CACHE = _build_nc()
    return _NC_CACHE


def _stage_inputs(position_ids, hidden_states, w_qkv, w_o):
    """Host-side sharding / layout staging. Returns in_maps for 8 cores."""
    pos = np.asarray(position_ids)[0].astype(np.float32)            # [S]
    hidden = np.asarray(hidden_states, dtype=np.float32)[0]         # [S, H]
    w_qkv = np.asarray(w_qkv, dtype=np.float32)                     # [3072, H]
    w_o = np.asarray(w_o, dtype=np.float32)                         # [H, Q_SIZE]

    # hT tiles: [H, S] -> [128, KT*S] (k-tile kt at cols kt*S..)
    hT = np.ascontiguousarray(hidden.T)
    hT_r = np.ascontiguousarray(
        hT.reshape(KT, P, S).transpose(1, 0, 2).reshape(P, KT * S)
    ).astype(BF16_NP)

    # w_o.T tiles: [Q_SIZE, H] -> [128, KT*H]
    woT = np.ascontiguousarray(w_o.T)
    woT_r = np.ascontiguousarray(
        woT.reshape(KT, P, H).transpose(1, 0, 2).reshape(P, KT * H)
    ).astype(BF16_NP)

    # rope tables in [d, s] layout for a [128 = 2 heads x 64] tile
    inv_freq = (1.0 / (ROPE_THETA ** (np.arange(0, HD, 2, dtype=np.float32) / HD)))
    ang = pos[:, None] * inv_freq[None, :]                          # [S, 32]
    cosT = np.cos(ang).T.astype(np.float32)                         # [32, S]
    sinT = np.sin(ang).T.astype(np.float32)
    ropec = np.concatenate([cosT, cosT, cosT, cosT], axis=0).astype(BF16_NP)
    ropes = np.concatenate([-sinT, sinT, -sinT, sinT], axis=0).astype(BF16_NP)

    # single [128, 128] lower-triangular mask: within any diagonal 128-block
    # d, live query column i is masked iff key partition p > i
    f = np.arange(P)
    masks = (np.arange(P)[:, None] <= f[None, :]).astype(BF16_NP)

    ident = np.eye(P, dtype=np.float32).astype(BF16_NP)

    in_maps = []
    for i in range(NCORES):
        rows_q = w_qkv[QH * HD * i : QH * HD * (i + 1)]             # [256, H]
        row_k = w_qkv[Q_SIZE + HD * i : Q_SIZE + HD * (i + 1)]      # [64, H]
        row_v = w_qkv[Q_SIZE + KV_SIZE + HD * i : Q_SIZE + KV_SIZE + HD * (i + 1)]
        wshard = np.concatenate([rows_q, row_k, row_v], axis=0)     # [384, H]
        wqkvT = np.ascontiguousarray(wshard.T)                      # [H, 384]
        wqkvT_r = np.ascontiguousarray(
            wqkvT.reshape(KT, P, 384).transpose(1, 0, 2).reshape(P, KT * 384)
        ).astype(BF16_NP)
        in_maps.append(
            {
                "hT": hT_r,
                "wqkvT": wqkvT_r,
                "woT": woT_r,
                "ropec": ropec,
                "ropes": ropes,
                "masks": masks,
                "ident": ident,
            }
        )
    return in_maps


def _ensure_ntff_hook():
    """The container's antenv stub lacks axon_hooks, so trn_boot silently
    skipped NTFF hook registration. Recreate the module and register the
    ctypes-based hook so run_bass_kernel_spmd(trace=True) can profile."""
    import sys
    import types

    if "antenv.axon_hooks" in sys.modules:
        return
    try:
        import antenv
        from trn_agent_boot.trn_boot import _ntff_profile_via_ctypes

        hooks = types.ModuleType("antenv.axon_hooks")
        _state = {}

        def set_axon_ntff_profile_hook(h):
            _state["h"] = h

        def get_axon_ntff_profile_hook():
            return _state.get("h")

        hooks.set_axon_ntff_profile_hook = set_axon_ntff_profile_hook
        hooks.get_axon_ntff_profile_hook = get_axon_ntff_profile_hook
        sys.modules["antenv.axon_hooks"] = hooks
        antenv.axon_hooks = hooks
        hook = _ntff_profile_via_ctypes("/opt/axon/libaxon_pjrt.so")
        if hook is not None:
            set_axon_ntff_profile_hook(hook)
    except Exception:
        pass


def kernel(**inputs):
    global LAST_RESULTS
    from concourse.bass_utils import run_bass_kernel_spmd

    nc = _get_nc()
    in_maps = _stage_inputs(
        inputs["position_ids"], inputs["hidden_states"], inputs["w_qkv"], inputs["w_o"]
    )
    trace = os.environ.get("KERNEL_TRACE", "0") == "1"
    if trace:
        _ensure_ntff_hook()
    res = run_bass_kernel_spmd(
        nc, in_maps, core_ids=list(range(NCORES)), trace=trace
    )
    LAST_RESULTS = res
    outs = [np.asarray(res.results[i]["out"], dtype=np.float32) for i in range(NCORES)]
    full = np.concatenate(outs, axis=0)                             # [S, H]
    return full.reshape(1, S, H)
